# revision 7
# baseline (speedup 1.0000x reference)
"""ContextAwareAttention Trainium2 kernel (v3).

Problem (hardcoded shapes): B=4, S=4096, DIM=256.
  q/k/v = complex linear projections of (z_real, z_imag); q gated by
  sigmoid(context @ wc.T + bc); scores = qf @ kf.T / 16; softmax;
  out = [attn @ v_r, attn @ v_i].

Sharding: 8 cores = 4 batches x 2 query-halves (2048 q rows each).
Host rolls z along the sequence axis per core so the kernel's q rows are
always rows 0..2047 (key-order permutation is softmax-invariant).

v3 design notes:
- Host pre-casts to bf16 AND pre-transposes z/ctx/weights, so the device
  does no casts and no transposes at all - feature-major tensors stream
  in with plain contiguous DMA loads.  (XBAR DMA transposes cost 1.26us
  of serialized HWDGE issue time each - 61us for z+ctx - and PE
  transposes burn tensor-engine time; host numpy is free.)
- k-projection eliminated algebraically: with p = gated q split into
  (p_r, p_i), scores = u_r . z_r^T + u_i . z_i^T where
  u_r = p_r Wkr + p_i Wki, u_i = p_i Wkr - p_r Wki.  u is [2048, 512]
  (q rows only), 4x cheaper to project than k over all 4096 keys.
- Softmax denominators: E_sum += e per key-chunk on the (otherwise idle)
  GpSimd engine; one 4-matmul partition-reduction per q-block instead of
  4 tiny matmuls per key-chunk (cuts 496 matmuls + LDWEIGHTS pressure).
- Tail: normalize muls split across DVE and ACT so the last q-block
  drains faster.
"""

import numpy as np
import ml_dtypes

import concourse.bass as bass
import concourse.mybir as mybir
import concourse.tile as tile
from concourse import bacc, bass_utils

F32 = mybir.dt.float32
BF16 = mybir.dt.bfloat16
BF16NP = ml_dtypes.bfloat16

B, S, D = 4, 4096, 256
D2 = 2 * D          # 512
SQ = S // 2         # 2048 q rows per core
SCALE = D ** (-0.5)
NCH = S // 512      # 8 phase-A chunks of 512 rows
KC = S // 128       # 32 key chunks
QB = SQ // 512      # 4 q blocks of 512


def _build():
    nc = bacc.Bacc("TRN2")
    # feature-major (transposed) bf16 inputs, host-prepped
    z_rT = nc.dram_tensor("z_rT", [D, S], BF16, kind="ExternalInput")
    z_iT = nc.dram_tensor("z_iT", [D, S], BF16, kind="ExternalInput")
    ctxT_d = nc.dram_tensor("ctxT", [D2, SQ], BF16, kind="ExternalInput")
    w_qrT = nc.dram_tensor("w_qrT", [D, D], BF16, kind="ExternalInput")
    w_qiT = nc.dram_tensor("w_qiT", [D, D], BF16, kind="ExternalInput")
    w_qinT = nc.dram_tensor("w_qinT", [D, D], BF16, kind="ExternalInput")
    w_kr = nc.dram_tensor("w_kr", [D, D], BF16, kind="ExternalInput")
    w_ki = nc.dram_tensor("w_ki", [D, D], BF16, kind="ExternalInput")
    w_kin = nc.dram_tensor("w_kin", [D, D], BF16, kind="ExternalInput")
    w_vzr = nc.dram_tensor("w_vzr", [D, D2], BF16, kind="ExternalInput")
    w_vzi = nc.dram_tensor("w_vzi", [D, D2], BF16, kind="ExternalInput")
    w_cT = nc.dram_tensor("w_cT", [D2, D2], BF16, kind="ExternalInput")
    b_c = nc.dram_tensor("b_c", [D2], F32, kind="ExternalInput")
    out = nc.dram_tensor("out", [SQ, D2], BF16, kind="ExternalOutput")

    mm = nc.tensor.matmul

    with tile.TileContext(nc) as tc:
        with tc.tile_pool(name="singles", bufs=1) as singles:
            ones = singles.tile([128, 1], BF16, tag="ones")
            nc.vector.memset(ones, 1.0)

            zT = singles.tile([128, 4, S], BF16, tag="zT")
            ctxT = singles.tile([128, 4, SQ], BF16, tag="ctxT")
            v = singles.tile([128, KC, D2], BF16, tag="v")
            qTg = singles.tile([128, 4, SQ], BF16, tag="qTg")
            uT = singles.tile([128, 4, SQ], BF16, tag="uT")

            # --- weights: host-prepped layouts, straight DMA loads.
            # v-path weights + first z chunks first so the PE starts ASAP.
            wsb = {}

            def load_w(name, w, nd):
                t = singles.tile([128, 2, 128 * nd], BF16, tag=f"w_{name}")
                nc.sync.dma_start(
                    out=t, in_=w.rearrange("(a p) d -> p a d", p=128))
                wsb[name] = t

            load_w("vzr", w_vzr, 4)
            load_w("vzi", w_vzi, 4)

            # ---- phase A: load transposed inputs + projections ----
            with (
                tc.tile_pool(name="gsb", bufs=2) as gsb,
                tc.tile_pool(name="pp", bufs=6, space="PSUM") as pp,
            ):
                for sc in range(NCH):
                    r0 = sc * 512
                    for dd in range(2):
                        nc.sync.dma_start(
                            out=zT[:, dd, r0:r0 + 512],
                            in_=z_rT[dd * 128:(dd + 1) * 128, r0:r0 + 512])
                        nc.sync.dma_start(
                            out=zT[:, 2 + dd, r0:r0 + 512],
                            in_=z_iT[dd * 128:(dd + 1) * 128, r0:r0 + 512])
                wcT = singles.tile([128, 4, D2], BF16, tag="wcT")
                nc.sync.dma_start(
                    out=wcT, in_=w_cT.rearrange("(a p) d -> p a d", p=128))
                bcT = singles.tile([128, 4], F32, tag="bcT")
                nc.sync.dma_start(
                    out=bcT, in_=b_c.rearrange("(c p) -> p c", p=128))
                for sc in range(NCH // 2):
                    r0 = sc * 512
                    for di in range(4):
                        nc.sync.dma_start(
                            out=ctxT[:, di, r0:r0 + 512],
                            in_=ctxT_d[di * 128:(di + 1) * 128, r0:r0 + 512])
                for name, w in (("qrT", w_qrT), ("qiT", w_qiT),
                                ("qinT", w_qinT), ("kr", w_kr),
                                ("ki", w_ki), ("kin", w_kin)):
                    load_w(name, w, 2)

                for sc in range(NCH):
                    r0 = sc * 512
                    # v rows for this chunk: [512, 512] in 4 psum subtiles
                    for a in range(4):
                        ps = pp.tile([128, 512], F32, tag="pp")
                        ra = r0 + a * 128
                        n = 0
                        for dd in range(2):
                            mm(ps, zT[:, dd, ra:ra + 128],
                               wsb["vzr"][:, dd, :], start=(n == 0),
                               stop=(n == 3))
                            n += 1
                            mm(ps, zT[:, 2 + dd, ra:ra + 128],
                               wsb["vzi"][:, dd, :], start=(n == 0),
                               stop=(n == 3))
                            n += 1
                        nc.vector.tensor_copy(out=v[:, sc * 4 + a, :], in_=ps)

                    if sc < NCH // 2:   # q rows: first 2048
                        for j in range(4):
                            gp = pp.tile([128, 512], F32, tag="pp")
                            for di in range(4):
                                mm(gp, wcT[:, di, j * 128:(j + 1) * 128],
                                   ctxT[:, di, r0:r0 + 512], start=(di == 0),
                                   stop=(di == 3))
                            gate = gsb.tile([128, 512], F32, tag="gate")
                            nc.scalar.activation(
                                out=gate, in_=gp,
                                func=mybir.ActivationFunctionType.Sigmoid,
                                bias=bcT[:, j:j + 1], scale=1.0)
                            qp = pp.tile([128, 512], F32, tag="pp")
                            jj = j % 2
                            if j < 2:   # q_r^T = Wqr z_r^T - Wqi z_i^T
                                terms = [("qrT", 0), ("qinT", 2)]
                            else:       # q_i^T = Wqr z_i^T + Wqi z_r^T
                                terms = [("qrT", 2), ("qiT", 0)]
                            n = 0
                            for wname, zoff in terms:
                                for dd in range(2):
                                    mm(qp,
                                       wsb[wname][:, dd,
                                                  jj * 128:(jj + 1) * 128],
                                       zT[:, zoff + dd, r0:r0 + 512],
                                       start=(n == 0), stop=(n == 3))
                                    n += 1
                            nc.vector.tensor_mul(
                                out=qTg[:, j, r0:r0 + 512], in0=qp, in1=gate)

                        # u^T for this q chunk (folds Wk into q side):
                        #   u_r = p_r Wkr + p_i Wki ; u_i = p_i Wkr - p_r Wki
                        for di in range(4):
                            up = pp.tile([128, 512], F32, tag="pp")
                            jj = di % 2
                            if di < 2:
                                terms = [("kr", 0), ("ki", 2)]
                            else:
                                terms = [("kr", 2), ("kin", 0)]
                            n = 0
                            for wname, qoff in terms:
                                for a in range(2):
                                    mm(up,
                                       wsb[wname][:, a,
                                                  jj * 128:(jj + 1) * 128],
                                       qTg[:, qoff + a, r0:r0 + 512],
                                       start=(n == 0), stop=(n == 3))
                                    n += 1
                            nc.vector.tensor_copy(
                                out=uT[:, di, r0:r0 + 512], in_=up)

            # ---- phase B: attention, single pass over all 32 key chunks ----
            with (
                tc.tile_pool(name="esb", bufs=3) as esb,
                tc.tile_pool(name="osb", bufs=3) as osb,
                tc.tile_pool(name="rcp", bufs=2) as rcp,
                tc.tile_pool(name="esum", bufs=2) as esump,
                tc.tile_pool(name="avsb", bufs=8) as avsb,
                tc.tile_pool(name="sps", bufs=3, space="PSUM") as sps,
                tc.tile_pool(name="avp", bufs=4, space="PSUM") as avp,
                tc.tile_pool(name="smp", bufs=1, space="PSUM") as smp,
            ):
                def emit_tail(qb, srcs, esb16):
                    # softmax denominators: one partition-reduction per qb.
                    # start=True clears has_written bank-wide, so only the
                    # first matmul starts; later columns' first writes rely
                    # on cleared bits (overwrite+set).
                    sm = smp.tile([128, 4], F32, tag="sm")
                    for qt in range(4):
                        mm(sm[:, qt:qt + 1],
                           esb16[:, qt * 128:(qt + 1) * 128], ones,
                           start=(qt == 0), stop=True)
                    r = rcp.tile([128, 4], F32, tag="r")
                    nc.vector.reciprocal(out=r, in_=sm)
                    for qt in range(4):
                        i = qb * 4 + qt
                        o = osb.tile([128, D2], BF16, tag="o")
                        if qt % 2 == 0:
                            nc.vector.tensor_scalar_mul(
                                out=o, in0=srcs[qt], scalar1=r[:, qt:qt + 1])
                            nc.sync.dma_start(
                                out=out[i * 128:(i + 1) * 128, :], in_=o)
                        else:
                            nc.scalar.activation(
                                out=o, in_=srcs[qt],
                                func=mybir.ActivationFunctionType.Copy,
                                scale=r[:, qt:qt + 1])
                            nc.scalar.dma_start(
                                out=out[i * 128:(i + 1) * 128, :], in_=o)

                pending = None
                for qb in range(QB):
                    av = [avp.tile([128, D2], F32, tag="av", name="av")
                          for _ in range(4)]
                    es = esump.tile([128, 512], F32, tag="es")
                    esb16 = esump.tile([128, 512], BF16, tag="esb16")
                    for kc in range(KC):
                        sp = sps.tile([128, 512], F32, tag="sp")
                        for di in range(4):
                            mm(sp, zT[:, di, kc * 128:(kc + 1) * 128],
                               uT[:, di, qb * 512:(qb + 1) * 512],
                               start=(di == 0), stop=(di == 3))
                        e = esb.tile([128, 512], BF16, tag="e")
                        nc.scalar.activation(
                            out=e, in_=sp,
                            func=mybir.ActivationFunctionType.Exp,
                            scale=float(SCALE))
                        for qt in range(4):
                            mm(av[qt], e[:, qt * 128:(qt + 1) * 128],
                               v[:, kc, :], start=(kc == 0),
                               stop=(kc == KC - 1))
                        if kc == 0:
                            nc.vector.tensor_copy(out=es, in_=e)
                        elif kc == KC - 1:
                            # last add emits the bf16 copy for the matmul
                            nc.vector.tensor_add(out=esb16, in0=es, in1=e)
                        else:
                            nc.vector.tensor_add(out=es, in0=es, in1=e)
                        if kc == 2 and pending is not None:
                            emit_tail(*pending)
                    if qb < QB - 1:
                        # free the PSUM banks for the next q-block; the
                        # normalize happens from SBUF inside its kc loop.
                        srcs = []
                        for qt in range(4):
                            a = avsb.tile([128, D2], F32, tag="avs")
                            nc.vector.tensor_copy(out=a, in_=av[qt])
                            srcs.append(a)
                    else:
                        srcs = av
                    pending = (qb, srcs, esb16)
                emit_tail(*pending)

    nc.finalize()
    return nc


_NC_CACHE = {}


def _bf16(x):
    return np.ascontiguousarray(np.asarray(x, dtype=np.float32)).astype(BF16NP)


def _bf16T(x):
    return np.ascontiguousarray(
        np.asarray(x, dtype=np.float32).T).astype(BF16NP)


def kernel(z_real, z_imag, context, wq_r, wq_i, wk_r, wk_i, wv_r, wv_i,
           wc, bc, _trace=False, _mm_dt=None):
    if "v3" not in _NC_CACHE:
        _NC_CACHE["v3"] = _build()
    nc = _NC_CACHE["v3"]

    z_real = np.asarray(z_real, dtype=np.float32)
    z_imag = np.asarray(z_imag, dtype=np.float32)
    context = np.asarray(context, dtype=np.float32)
    f32 = lambda x: np.ascontiguousarray(np.asarray(x, dtype=np.float32))
    wq_r, wq_i = f32(wq_r), f32(wq_i)
    wk_r, wk_i = f32(wk_r), f32(wk_i)
    wv_r, wv_i = f32(wv_r), f32(wv_i)
    wc_, bc_ = f32(wc), f32(bc)

    ws = {
        "w_qrT": _bf16T(wq_r),
        "w_qiT": _bf16T(wq_i),
        "w_qinT": _bf16T(-wq_i),
        "w_kr": _bf16(wk_r),
        "w_ki": _bf16(wk_i),
        "w_kin": _bf16(-wk_i),
        "w_vzr": _bf16(np.concatenate([wv_r.T, wv_i.T], axis=1)),
        "w_vzi": _bf16(np.concatenate([-wv_i.T, wv_r.T], axis=1)),
        "w_cT": _bf16T(wc_),
        "b_c": bc_,
    }

    in_maps = []
    for c in range(8):
        b, h = c // 2, c % 2
        in_maps.append({
            "z_rT": _bf16T(np.roll(z_real[b], -h * SQ, axis=0)),
            "z_iT": _bf16T(np.roll(z_imag[b], -h * SQ, axis=0)),
            "ctxT": _bf16T(context[b, h * SQ:(h + 1) * SQ]),
            **ws,
        })
    res = bass_utils.run_bass_kernel_spmd(
        nc, in_maps, core_ids=list(range(8)), trace=_trace)

    full = np.empty((B, S, D2), dtype=np.float32)
    for c in range(8):
        b, h = c // 2, c % 2
        full[b, h * SQ:(h + 1) * SQ, :] = res.results[c]["out"]
    if _trace:
        return full, res
    return full


# revision 8
# speedup vs baseline: 1.0542x; 1.0542x over previous
"""ContextAwareAttention Trainium2 kernel (v5).

Problem (hardcoded shapes): B=4, S=4096, DIM=256.
  q/k/v = complex linear projections of (z_real, z_imag); q gated by
  sigmoid(context @ wc.T + bc); scores = qf @ kf.T / 16; softmax;
  out = [attn @ v_r, attn @ v_i].

Sharding: 8 cores = 4 batches x 2 query-halves (2048 q rows each).
Host rolls z along the sequence axis per core so the kernel's q rows are
always rows 0..2047 (key-order permutation is softmax-invariant).

v5 design notes:
- Host pre-casts to bf16, pre-transposes, and lays every input out as
  the exact SBUF partition image [128, ...], so each DMA moves >=3KB of
  contiguous bytes per partition (DMA descriptors cost ~40ns each
  regardless of size; 1KB-segment loads were descriptor-bound).
- k-projection eliminated algebraically: with p = gated q split into
  (p_r, p_i), scores = u_r . z_r^T + u_i . z_i^T where
  u_r = p_r Wkr + p_i Wki, u_i = p_i Wkr - p_r Wki.  u is [2048, 512]
  (q rows only), 4x cheaper to project than k over all 4096 keys.
- Softmax denominators: DVE accumulates E_sum += e per key-chunk; one
  4-matmul partition-reduction per q-block instead of 4 tiny matmuls
  per key-chunk.
- q-block tails (denominator matmuls + normalize + store) are deferred
  into the next q-block's key loop so the PE never waits on them; av
  PSUM banks are freed via copies to SBUF.
- Output is written bf16 as a [128, 16, 512] partition image (pairs of
  row-blocks merged per DMA); host unpacks/casts to the f32 result.
"""

import numpy as np
import ml_dtypes

import concourse.bass as bass
import concourse.mybir as mybir
import concourse.tile as tile
from concourse import bacc, bass_utils

F32 = mybir.dt.float32
BF16 = mybir.dt.bfloat16
BF16NP = ml_dtypes.bfloat16

B, S, D = 4, 4096, 256
D2 = 2 * D          # 512
SQ = S // 2         # 2048 q rows per core
SCALE = D ** (-0.5)
NCH = S // 512      # 8 phase-A chunks of 512 rows
KC = S // 128       # 32 key chunks
QB = SQ // 512      # 4 q blocks of 512


def _build():
    nc = bacc.Bacc("TRN2")
    # All inputs are host-prepped bf16 SBUF partition images.
    z_img = nc.dram_tensor("z_img", [128, 4, S], BF16, kind="ExternalInput")
    ctx_img = nc.dram_tensor("ctx_img", [128, 4, SQ], BF16,
                             kind="ExternalInput")
    wq_img = nc.dram_tensor("wq_img", [128, 3, 2, D], BF16,
                            kind="ExternalInput")
    wk_img = nc.dram_tensor("wk_img", [128, 3, 2, D], BF16,
                            kind="ExternalInput")
    wv_img = nc.dram_tensor("wv_img", [128, 2, 2, D2], BF16,
                            kind="ExternalInput")
    wc_img = nc.dram_tensor("wc_img", [128, 4, D2], BF16,
                            kind="ExternalInput")
    bc_img = nc.dram_tensor("bc_img", [128, 4], F32, kind="ExternalInput")
    out = nc.dram_tensor("out", [128, 16, D2], BF16, kind="ExternalOutput")

    mm = nc.tensor.matmul
    QR, QI, QIN = 0, 1, 2    # wq_img slots
    KR, KI, KIN = 0, 1, 2    # wk_img slots
    VZR, VZI = 0, 1          # wv_img slots

    with tile.TileContext(nc) as tc:
        with tc.tile_pool(name="singles", bufs=1) as singles:
            ones = singles.tile([128, 1], BF16, tag="ones")
            nc.vector.memset(ones, 1.0)

            zT = singles.tile([128, 4, S], BF16, tag="zT")
            ctxT = singles.tile([128, 4, SQ], BF16, tag="ctxT")
            v = singles.tile([128, KC, D2], BF16, tag="v")
            qTg = singles.tile([128, 4, SQ], BF16, tag="qTg")
            uT = singles.tile([128, 4, SQ], BF16, tag="uT")

            # v-path weights + z first so the PE starts ASAP.
            wv = singles.tile([128, 2, 2, D2], BF16, tag="wv")
            nc.sync.dma_start(out=wv, in_=wv_img[:])
            for di in range(4):
                for h in range(2):
                    nc.sync.dma_start(
                        out=zT[:, di, h * 2048:(h + 1) * 2048],
                        in_=z_img[:, di, h * 2048:(h + 1) * 2048])
            wc = singles.tile([128, 4, D2], BF16, tag="wc")
            nc.sync.dma_start(out=wc, in_=wc_img[:])
            bcT = singles.tile([128, 4], F32, tag="bcT")
            nc.sync.dma_start(out=bcT, in_=bc_img[:])
            for di in range(4):
                nc.sync.dma_start(out=ctxT[:, di, :], in_=ctx_img[:, di, :])
            wq = singles.tile([128, 3, 2, D], BF16, tag="wq")
            nc.sync.dma_start(out=wq, in_=wq_img[:])
            wk = singles.tile([128, 3, 2, D], BF16, tag="wk")
            nc.sync.dma_start(out=wk, in_=wk_img[:])

            # ---- phase A: projections ----
            with (
                tc.tile_pool(name="gsb", bufs=2) as gsb,
                tc.tile_pool(name="pp", bufs=6, space="PSUM") as pp,
            ):
                for sc in range(NCH):
                    r0 = sc * 512
                    # v rows for this chunk: [512, 512] in 4 psum subtiles
                    for a in range(4):
                        ps = pp.tile([128, 512], F32, tag="pp")
                        ra = r0 + a * 128
                        n = 0
                        for dd in range(2):
                            mm(ps, zT[:, dd, ra:ra + 128],
                               wv[:, VZR, dd, :], start=(n == 0),
                               stop=(n == 3))
                            n += 1
                            mm(ps, zT[:, 2 + dd, ra:ra + 128],
                               wv[:, VZI, dd, :], start=(n == 0),
                               stop=(n == 3))
                            n += 1
                        nc.vector.tensor_copy(out=v[:, sc * 4 + a, :], in_=ps)

                    if sc < NCH // 2:   # q rows: first 2048
                        for j in range(4):
                            gp = pp.tile([128, 512], F32, tag="pp")
                            for di in range(4):
                                mm(gp, wc[:, di, j * 128:(j + 1) * 128],
                                   ctxT[:, di, r0:r0 + 512], start=(di == 0),
                                   stop=(di == 3))
                            gate = gsb.tile([128, 512], F32, tag="gate")
                            nc.scalar.activation(
                                out=gate, in_=gp,
                                func=mybir.ActivationFunctionType.Sigmoid,
                                bias=bcT[:, j:j + 1], scale=1.0)
                            qp = pp.tile([128, 512], F32, tag="pp")
                            jj = j % 2
                            if j < 2:   # q_r^T = Wqr z_r^T - Wqi z_i^T
                                terms = [(QR, 0), (QIN, 2)]
                            else:       # q_i^T = Wqr z_i^T + Wqi z_r^T
                                terms = [(QR, 2), (QI, 0)]
                            n = 0
                            for widx, zoff in terms:
                                for dd in range(2):
                                    mm(qp,
                                       wq[:, widx, dd,
                                          jj * 128:(jj + 1) * 128],
                                       zT[:, zoff + dd, r0:r0 + 512],
                                       start=(n == 0), stop=(n == 3))
                                    n += 1
                            nc.vector.tensor_mul(
                                out=qTg[:, j, r0:r0 + 512], in0=qp, in1=gate)

                        # u^T for this q chunk (folds Wk into q side):
                        #   u_r = p_r Wkr + p_i Wki ; u_i = p_i Wkr - p_r Wki
                        for di in range(4):
                            up = pp.tile([128, 512], F32, tag="pp")
                            jj = di % 2
                            if di < 2:
                                terms = [(KR, 0), (KI, 2)]
                            else:
                                terms = [(KR, 2), (KIN, 0)]
                            n = 0
                            for widx, qoff in terms:
                                for a in range(2):
                                    mm(up,
                                       wk[:, widx, a,
                                          jj * 128:(jj + 1) * 128],
                                       qTg[:, qoff + a, r0:r0 + 512],
                                       start=(n == 0), stop=(n == 3))
                                    n += 1
                            nc.vector.tensor_copy(
                                out=uT[:, di, r0:r0 + 512], in_=up)

            # ---- phase B: attention, single pass over all 32 key chunks ----
            with (
                tc.tile_pool(name="esb", bufs=3) as esb,
                tc.tile_pool(name="osb", bufs=2) as osb,
                tc.tile_pool(name="rcp", bufs=2) as rcp,
                tc.tile_pool(name="esum", bufs=2) as esump,
                tc.tile_pool(name="avsb", bufs=8) as avsb,
                tc.tile_pool(name="sps", bufs=3, space="PSUM") as sps,
                tc.tile_pool(name="avp", bufs=4, space="PSUM") as avp,
                tc.tile_pool(name="smp", bufs=1, space="PSUM") as smp,
            ):
                def emit_tail(qb, srcs, esb16):
                    # softmax denominators: one partition-reduction per qb.
                    # start=True clears has_written bank-wide, so only the
                    # first matmul starts; later columns' first writes rely
                    # on cleared bits (overwrite+set).
                    sm = smp.tile([128, 4], F32, tag="sm")
                    for qt in range(4):
                        mm(sm[:, qt:qt + 1],
                           esb16[:, qt * 128:(qt + 1) * 128], ones,
                           start=(qt == 0), stop=True)
                    r = rcp.tile([128, 4], F32, tag="r")
                    nc.vector.reciprocal(out=r, in_=sm)
                    # out image slot: m = qb*4 + 2*(qt%2) + qt//2 pairs the
                    # two DVE-normalized blocks (and the two ACT ones) into
                    # adjacent slots -> one 2KB-per-partition DMA each.
                    o_ev = osb.tile([128, 2, D2], BF16, tag="o_ev")
                    o_od = osb.tile([128, 2, D2], BF16, tag="o_od")
                    for qt in range(4):
                        dst = (o_ev if qt % 2 == 0 else o_od)[:, qt // 2, :]
                        if qt % 2 == 0:
                            nc.vector.tensor_scalar_mul(
                                out=dst, in0=srcs[qt], scalar1=r[:, qt:qt + 1])
                        else:
                            nc.scalar.activation(
                                out=dst, in_=srcs[qt],
                                func=mybir.ActivationFunctionType.Copy,
                                scale=r[:, qt:qt + 1])
                    nc.sync.dma_start(
                        out=out[:, qb * 4:qb * 4 + 2, :], in_=o_ev)
                    nc.scalar.dma_start(
                        out=out[:, qb * 4 + 2:qb * 4 + 4, :], in_=o_od)

                pending = None
                for qb in range(QB):
                    av = [avp.tile([128, D2], F32, tag="av", name="av")
                          for _ in range(4)]
                    es = esump.tile([128, 512], F32, tag="es")
                    esb16 = esump.tile([128, 512], BF16, tag="esb16")
                    for kc in range(KC):
                        sp = sps.tile([128, 512], F32, tag="sp")
                        for di in range(4):
                            mm(sp, zT[:, di, kc * 128:(kc + 1) * 128],
                               uT[:, di, qb * 512:(qb + 1) * 512],
                               start=(di == 0), stop=(di == 3))
                        e = esb.tile([128, 512], BF16, tag="e")
                        nc.scalar.activation(
                            out=e, in_=sp,
                            func=mybir.ActivationFunctionType.Exp,
                            scale=float(SCALE))
                        for qt in range(4):
                            mm(av[qt], e[:, qt * 128:(qt + 1) * 128],
                               v[:, kc, :], start=(kc == 0),
                               stop=(kc == KC - 1))
                        if kc == 0:
                            nc.vector.tensor_copy(out=es, in_=e)
                        elif kc == KC - 1:
                            # last add emits the bf16 copy for the matmul
                            nc.vector.tensor_add(out=esb16, in0=es, in1=e)
                        else:
                            nc.vector.tensor_add(out=es, in0=es, in1=e)
                        if kc == 2 and pending is not None:
                            emit_tail(*pending)
                    if qb < QB - 1:
                        # free the PSUM banks for the next q-block; the
                        # normalize happens from SBUF inside its kc loop.
                        srcs = []
                        for qt in range(4):
                            a = avsb.tile([128, D2], F32, tag="avs")
                            nc.vector.tensor_copy(out=a, in_=av[qt])
                            srcs.append(a)
                    else:
                        srcs = av
                    pending = (qb, srcs, esb16)
                emit_tail(*pending)

    nc.finalize()
    return nc


_NC_CACHE = {}


def _img(m):
    """[X*128, Y] f32 -> bf16 SBUF partition image [128, X, Y]."""
    x, y = m.shape
    return np.ascontiguousarray(
        m.reshape(x // 128, 128, y).transpose(1, 0, 2)).astype(BF16NP)


def kernel(z_real, z_imag, context, wq_r, wq_i, wk_r, wk_i, wv_r, wv_i,
           wc, bc, _trace=False, _mm_dt=None):
    if "v5" not in _NC_CACHE:
        _NC_CACHE["v5"] = _build()
    nc = _NC_CACHE["v5"]

    z_real = np.asarray(z_real, dtype=np.float32)
    z_imag = np.asarray(z_imag, dtype=np.float32)
    context = np.asarray(context, dtype=np.float32)
    f32 = lambda x: np.ascontiguousarray(np.asarray(x, dtype=np.float32))
    wq_r, wq_i = f32(wq_r), f32(wq_i)
    wk_r, wk_i = f32(wk_r), f32(wk_i)
    wv_r, wv_i = f32(wv_r), f32(wv_i)
    wc_, bc_ = f32(wc), f32(bc)

    ws = {
        "wq_img": np.stack(
            [_img(wq_r.T), _img(wq_i.T), _img(-wq_i.T)], axis=1),
        "wk_img": np.stack(
            [_img(wk_r), _img(wk_i), _img(-wk_i)], axis=1),
        "wv_img": np.stack(
            [_img(np.ascontiguousarray(np.concatenate(
                [wv_r.T, wv_i.T], axis=1))),
             _img(np.ascontiguousarray(np.concatenate(
                 [-wv_i.T, wv_r.T], axis=1)))], axis=1),
        "wc_img": _img(np.ascontiguousarray(wc_.T)),
        "bc_img": np.ascontiguousarray(
            bc_.reshape(4, 128).T),
    }

    in_maps = []
    for c in range(8):
        b, h = c // 2, c % 2
        zr = np.roll(z_real[b], -h * SQ, axis=0)
        zi = np.roll(z_imag[b], -h * SQ, axis=0)
        cx = context[b, h * SQ:(h + 1) * SQ]
        in_maps.append({
            "z_img": np.concatenate(
                [_img(np.ascontiguousarray(zr.T)),
                 _img(np.ascontiguousarray(zi.T))], axis=1),
            "ctx_img": _img(np.ascontiguousarray(cx.T)),
            **ws,
        })
    res = bass_utils.run_bass_kernel_spmd(
        nc, in_maps, core_ids=list(range(8)), trace=_trace)

    # unpack the out image: slot m = qb*4 + 2*(qt%2) + qt//2
    full = np.empty((B, S, D2), dtype=np.float32)
    for c in range(8):
        b, h = c // 2, c % 2
        img = np.asarray(res.results[c]["out"], dtype=np.float32)
        dst = full[b, h * SQ:(h + 1) * SQ, :]
        for m in range(16):
            qb, u = divmod(m, 4)
            qt = 2 * (u % 2) + u // 2
            i = qb * 4 + qt
            dst[i * 128:(i + 1) * 128, :] = img[:, m, :]
    if _trace:
        return full, res
    return full


# revision 10
# speedup vs baseline: 1.1074x; 1.0504x over previous
"""ContextAwareAttention Trainium2 kernel (v6).

Problem (hardcoded shapes): B=4, S=4096, DIM=256.
  q/k/v = complex linear projections of (z_real, z_imag); q gated by
  sigmoid(context @ wc.T + bc); scores = qf @ kf.T / 16; softmax;
  out = [attn @ v_r, attn @ v_i].

Sharding: 8 cores = 4 batches x 2 query-halves (2048 q rows each).
Host rolls z along the sequence axis per core so the kernel's q rows are
always rows 0..2047 (key-order permutation is softmax-invariant).

v6 design notes:
- Host pre-casts to bf16, pre-transposes, and lays every input out as
  the exact SBUF partition image [128, ...], so each DMA moves >=2KB of
  contiguous bytes per partition (descriptors cost ~40ns each
  regardless of size; 1KB-segment loads were descriptor-bound).
- k-projection eliminated algebraically: with p = gated q split into
  (p_r, p_i), scores = u_r . z_r^T + u_i . z_i^T where
  u_r = p_r Wkr + p_i Wki, u_i = p_i Wkr - p_r Wki.  u covers q rows
  only - 4x cheaper to project than k over all 4096 keys.
- v-projection eliminated the same way on the output side: the AV step
  accumulates out1 = z^T e (same matmul count as attn @ v), and the
  tiny per-q-block post-projection out = out1^T [Wv...] replaces the
  full-key v projection: 64 matmuls instead of 128.
- Softmax denominators: DVE accumulates E_sum += e per key-chunk; one
  4-matmul partition-reduction per q-block instead of 4 tiny matmuls
  per key-chunk.
- q-block tails (denominator matmuls + post-projection + normalize +
  store) are deferred into the next q-block's key loop so the PE never
  idles on them; out1 PSUM banks are freed via copies to SBUF.
- A 16-matmul warmup burst on zeroed SBUF runs during the DMA preamble
  so the PE's HAM clock-gate is already at 8/8 when real work arrives.
- Output is written bf16 as a [128, 16, 512] partition image (pairs of
  row-blocks merged per DMA); host unpacks/casts to the f32 result.
"""

import numpy as np
import ml_dtypes

import concourse.bass as bass
import concourse.mybir as mybir
import concourse.tile as tile
from concourse import bacc, bass_utils

F32 = mybir.dt.float32
BF16 = mybir.dt.bfloat16
BF16NP = ml_dtypes.bfloat16

B, S, D = 4, 4096, 256
D2 = 2 * D          # 512
SQ = S // 2         # 2048 q rows per core
SCALE = D ** (-0.5)
NCH = SQ // 512     # 4 phase-A q chunks of 512 rows
KC = S // 128       # 32 key chunks
QB = SQ // 512      # 4 q blocks of 512


def _build():
    nc = bacc.Bacc("TRN2")
    # All inputs are host-prepped bf16 SBUF partition images.
    z_img = nc.dram_tensor("z_img", [128, 4, S], BF16, kind="ExternalInput")
    zr_img = nc.dram_tensor("zr_img", [128, KC, D2], BF16,
                            kind="ExternalInput")
    ctx_img = nc.dram_tensor("ctx_img", [128, 4, SQ], BF16,
                             kind="ExternalInput")
    wq_img = nc.dram_tensor("wq_img", [128, 3, 2, D], BF16,
                            kind="ExternalInput")
    wk_img = nc.dram_tensor("wk_img", [128, 3, 2, D], BF16,
                            kind="ExternalInput")
    wv_img = nc.dram_tensor("wv_img", [128, 2, 2, D2], BF16,
                            kind="ExternalInput")
    wc_img = nc.dram_tensor("wc_img", [128, 4, D2], BF16,
                            kind="ExternalInput")
    bc_img = nc.dram_tensor("bc_img", [128, 4], F32, kind="ExternalInput")
    out = nc.dram_tensor("out", [128, 16, D2], BF16, kind="ExternalOutput")

    mm = nc.tensor.matmul
    QR, QI, QIN = 0, 1, 2    # wq_img slots
    KR, KI, KIN = 0, 1, 2    # wk_img slots
    VZR, VZI = 0, 1          # wv_img slots

    with tile.TileContext(nc) as tc:
        with tc.tile_pool(name="singles", bufs=1) as singles:
            ones = singles.tile([128, 1], BF16, tag="ones")
            nc.vector.memset(ones, 1.0)

            zT = singles.tile([128, 4, S], BF16, tag="zT")
            zR = singles.tile([128, KC, D2], BF16, tag="zR")
            ctxT = singles.tile([128, 4, SQ], BF16, tag="ctxT")
            qTg = singles.tile([128, 4, SQ], BF16, tag="qTg")
            uT = singles.tile([128, 4, SQ], BF16, tag="uT")

            # gate-path weights + q-row z first so the PE starts ASAP.
            wc = singles.tile([128, 4, D2], BF16, tag="wc")
            nc.sync.dma_start(out=wc, in_=wc_img[:])
            bcT = singles.tile([128, 4], F32, tag="bcT")
            nc.sync.dma_start(out=bcT, in_=bc_img[:])
            for di in range(4):
                nc.sync.dma_start(out=ctxT[:, di, :], in_=ctx_img[:, di, :])
                nc.sync.dma_start(
                    out=zT[:, di, 0:2048], in_=z_img[:, di, 0:2048])
            wq = singles.tile([128, 3, 2, D], BF16, tag="wq")
            nc.sync.dma_start(out=wq, in_=wq_img[:])
            wk = singles.tile([128, 3, 2, D], BF16, tag="wk")
            nc.sync.dma_start(out=wk, in_=wk_img[:])
            wv = singles.tile([128, 2, 2, D2], BF16, tag="wv")
            nc.sync.dma_start(out=wv, in_=wv_img[:])
            for di in range(4):
                nc.sync.dma_start(
                    out=zT[:, di, 2048:4096], in_=z_img[:, di, 2048:4096])
            for g in range(4):
                nc.sync.dma_start(
                    out=zR[:, g * 8:(g + 1) * 8, :],
                    in_=zr_img[:, g * 8:(g + 1) * 8, :])

            # ---- phase A: gate/q/u projections over the 2048 q rows ----
            with (
                tc.tile_pool(name="gsb", bufs=2) as gsb,
                tc.tile_pool(name="pp", bufs=6, space="PSUM") as pp,
            ):
                # HAM warmup: harmless matmuls on zeroed SBUF while the
                # input DMAs land, so real matmuls start at full clock.
                wu = singles.tile([128, 512], BF16, tag="wu")
                nc.gpsimd.memset(wu, 0)
                for _ in range(16):
                    wup = pp.tile([128, 512], F32, tag="pp")
                    mm(wup, wu[:, 0:128], wu, start=True, stop=True)

                for sc in range(NCH):
                    r0 = sc * 512
                    for j in range(4):
                        gp = pp.tile([128, 512], F32, tag="pp")
                        for di in range(4):
                            mm(gp, wc[:, di, j * 128:(j + 1) * 128],
                               ctxT[:, di, r0:r0 + 512], start=(di == 0),
                               stop=(di == 3))
                        gate = gsb.tile([128, 512], F32, tag="gate")
                        nc.scalar.activation(
                            out=gate, in_=gp,
                            func=mybir.ActivationFunctionType.Sigmoid,
                            bias=bcT[:, j:j + 1], scale=1.0)
                        qp = pp.tile([128, 512], F32, tag="pp")
                        jj = j % 2
                        if j < 2:   # q_r^T = Wqr z_r^T - Wqi z_i^T
                            terms = [(QR, 0), (QIN, 2)]
                        else:       # q_i^T = Wqr z_i^T + Wqi z_r^T
                            terms = [(QR, 2), (QI, 0)]
                        n = 0
                        for widx, zoff in terms:
                            for dd in range(2):
                                mm(qp,
                                   wq[:, widx, dd, jj * 128:(jj + 1) * 128],
                                   zT[:, zoff + dd, r0:r0 + 512],
                                   start=(n == 0), stop=(n == 3))
                                n += 1
                        nc.vector.tensor_mul(
                            out=qTg[:, j, r0:r0 + 512], in0=qp, in1=gate)

                    # u^T for this q chunk (folds Wk into q side):
                    #   u_r = p_r Wkr + p_i Wki ; u_i = p_i Wkr - p_r Wki
                    for di in range(4):
                        up = pp.tile([128, 512], F32, tag="pp")
                        jj = di % 2
                        if di < 2:
                            terms = [(KR, 0), (KI, 2)]
                        else:
                            terms = [(KR, 2), (KIN, 0)]
                        n = 0
                        for widx, qoff in terms:
                            for a in range(2):
                                mm(up,
                                   wk[:, widx, a, jj * 128:(jj + 1) * 128],
                                   qTg[:, qoff + a, r0:r0 + 512],
                                   start=(n == 0), stop=(n == 3))
                                n += 1
                        nc.vector.tensor_copy(
                            out=uT[:, di, r0:r0 + 512], in_=up)

            # ---- phase B: attention, single pass over all 32 key chunks ----
            with (
                tc.tile_pool(name="esb", bufs=3) as esb,
                tc.tile_pool(name="osb", bufs=2) as osb,
                tc.tile_pool(name="rcp", bufs=2) as rcp,
                tc.tile_pool(name="esum", bufs=2) as esump,
                tc.tile_pool(name="o1sb", bufs=8) as o1sb,
                tc.tile_pool(name="sps", bufs=3, space="PSUM") as sps,
                tc.tile_pool(name="o1p", bufs=4, space="PSUM") as o1p,
                tc.tile_pool(name="smp", bufs=1, space="PSUM") as smp,
            ):
                def emit_tail(qb, o1s, esb16):
                    # softmax denominators: one partition-reduction per qb.
                    # start=True clears has_written bank-wide, so only the
                    # first matmul starts; later columns' first writes rely
                    # on cleared bits (overwrite+set).
                    sm = smp.tile([128, 4], F32, tag="sm")
                    for qt in range(4):
                        mm(sm[:, qt:qt + 1],
                           esb16[:, qt * 128:(qt + 1) * 128], ones,
                           start=(qt == 0), stop=True)
                    r = rcp.tile([128, 4], F32, tag="r")
                    nc.vector.reciprocal(out=r, in_=sm)
                    # post-projection: out rows = out1^T [Wv combined],
                    # then normalize by 1/rowsum and store.
                    # out image slot m = qb*4 + 2*(qt%2) + qt//2 pairs the
                    # two DVE-normalized blocks (and the two ACT ones) into
                    # adjacent slots -> one 2KB-per-partition DMA each.
                    o_ev = osb.tile([128, 2, D2], BF16, tag="o_ev")
                    o_od = osb.tile([128, 2, D2], BF16, tag="o_od")
                    for qt in range(4):
                        prj = sps.tile([128, 512], F32, tag="sp")
                        n = 0
                        for dd in range(2):
                            mm(prj, o1s[dd][:, qt * 128:(qt + 1) * 128],
                               wv[:, VZR, dd, :], start=(n == 0),
                               stop=(n == 3))
                            n += 1
                            mm(prj, o1s[2 + dd][:, qt * 128:(qt + 1) * 128],
                               wv[:, VZI, dd, :], start=(n == 0),
                               stop=(n == 3))
                            n += 1
                        dst = (o_ev if qt % 2 == 0 else o_od)[:, qt // 2, :]
                        if qt % 2 == 0:
                            nc.vector.tensor_scalar_mul(
                                out=dst, in0=prj, scalar1=r[:, qt:qt + 1])
                        else:
                            nc.scalar.activation(
                                out=dst, in_=prj,
                                func=mybir.ActivationFunctionType.Copy,
                                scale=r[:, qt:qt + 1])
                    nc.sync.dma_start(
                        out=out[:, qb * 4:qb * 4 + 2, :], in_=o_ev)
                    nc.scalar.dma_start(
                        out=out[:, qb * 4 + 2:qb * 4 + 4, :], in_=o_od)

                pending = None
                for qb in range(QB):
                    out1 = [o1p.tile([128, D2], F32, tag="out1", name="out1")
                            for _ in range(4)]
                    es = esump.tile([128, 512], F32, tag="es")
                    esb16 = esump.tile([128, 512], BF16, tag="esb16")
                    for kc in range(KC):
                        sp = sps.tile([128, 512], F32, tag="sp")
                        for di in range(4):
                            mm(sp, zT[:, di, kc * 128:(kc + 1) * 128],
                               uT[:, di, qb * 512:(qb + 1) * 512],
                               start=(di == 0), stop=(di == 3))
                        e = esb.tile([128, 512], BF16, tag="e")
                        nc.scalar.activation(
                            out=e, in_=sp,
                            func=mybir.ActivationFunctionType.Exp,
                            scale=float(SCALE))
                        for dc in range(4):
                            mm(out1[dc], zR[:, kc, dc * 128:(dc + 1) * 128],
                               e, start=(kc == 0), stop=(kc == KC - 1))
                        if kc == 0:
                            nc.vector.tensor_copy(out=es, in_=e)
                        elif kc == KC - 1:
                            # last add emits the bf16 copy for the matmul
                            nc.vector.tensor_add(out=esb16, in0=es, in1=e)
                        else:
                            nc.vector.tensor_add(out=es, in0=es, in1=e)
                        if kc == 2 and pending is not None:
                            emit_tail(*pending)
                    # free out1 PSUM banks: copy to SBUF (bf16) for the
                    # post-projection, split across DVE and GpSimd.
                    o1s = []
                    for dc in range(4):
                        a = o1sb.tile([128, D2], BF16, tag="o1s")
                        if dc % 2 == 0:
                            nc.vector.tensor_copy(out=a, in_=out1[dc])
                        else:
                            nc.scalar.activation(
                                out=a, in_=out1[dc],
                                func=mybir.ActivationFunctionType.Copy)
                        o1s.append(a)
                    pending = (qb, o1s, esb16)
                emit_tail(*pending)

    nc.finalize()
    return nc


_NC_CACHE = {}


def _img(m):
    """[X*128, Y] f32 -> bf16 SBUF partition image [128, X, Y]."""
    x, y = m.shape
    return np.ascontiguousarray(
        m.reshape(x // 128, 128, y).transpose(1, 0, 2)).astype(BF16NP)


def kernel(z_real, z_imag, context, wq_r, wq_i, wk_r, wk_i, wv_r, wv_i,
           wc, bc, _trace=False, _mm_dt=None):
    if "v6" not in _NC_CACHE:
        _NC_CACHE["v6"] = _build()
    nc = _NC_CACHE["v6"]

    z_real = np.asarray(z_real, dtype=np.float32)
    z_imag = np.asarray(z_imag, dtype=np.float32)
    context = np.asarray(context, dtype=np.float32)
    f32 = lambda x: np.ascontiguousarray(np.asarray(x, dtype=np.float32))
    wq_r, wq_i = f32(wq_r), f32(wq_i)
    wk_r, wk_i = f32(wk_r), f32(wk_i)
    wv_r, wv_i = f32(wv_r), f32(wv_i)
    wc_, bc_ = f32(wc), f32(bc)

    ws = {
        "wq_img": np.stack(
            [_img(wq_r.T), _img(wq_i.T), _img(-wq_i.T)], axis=1),
        "wk_img": np.stack(
            [_img(wk_r), _img(wk_i), _img(-wk_i)], axis=1),
        "wv_img": np.stack(
            [_img(np.ascontiguousarray(np.concatenate(
                [wv_r.T, wv_i.T], axis=1))),
             _img(np.ascontiguousarray(np.concatenate(
                 [-wv_i.T, wv_r.T], axis=1)))], axis=1),
        "wc_img": _img(np.ascontiguousarray(wc_.T)),
        "bc_img": np.ascontiguousarray(
            bc_.reshape(4, 128).T),
    }

    in_maps = []
    for c in range(8):
        b, h = c // 2, c % 2
        zr = np.roll(z_real[b], -h * SQ, axis=0)
        zi = np.roll(z_imag[b], -h * SQ, axis=0)
        cx = context[b, h * SQ:(h + 1) * SQ]
        in_maps.append({
            "z_img": np.concatenate(
                [_img(np.ascontiguousarray(zr.T)),
                 _img(np.ascontiguousarray(zi.T))], axis=1),
            "zr_img": _img(np.concatenate([zr, zi], axis=1)),
            "ctx_img": _img(np.ascontiguousarray(cx.T)),
            **ws,
        })
    res = bass_utils.run_bass_kernel_spmd(
        nc, in_maps, core_ids=list(range(8)), trace=_trace)

    # unpack the out image: slot m = qb*4 + 2*(qt%2) + qt//2
    full = np.empty((B, S, D2), dtype=np.float32)
    for c in range(8):
        b, h = c // 2, c % 2
        img = np.asarray(res.results[c]["out"], dtype=np.float32)
        dst = full[b, h * SQ:(h + 1) * SQ, :]
        for m in range(16):
            qb, u = divmod(m, 4)
            qt = 2 * (u % 2) + u // 2
            i = qb * 4 + qt
            dst[i * 128:(i + 1) * 128, :] = img[:, m, :]
    if _trace:
        return full, res
    return full


# revision 11
# speedup vs baseline: 1.1397x; 1.0292x over previous
"""ContextAwareAttention Trainium2 kernel (v7).

Problem (hardcoded shapes): B=4, S=4096, DIM=256.
  q/k/v = complex linear projections of (z_real, z_imag); q gated by
  sigmoid(context @ wc.T + bc); scores = qf @ kf.T / 16; softmax;
  out = [attn @ v_r, attn @ v_i].

Sharding: 8 cores = 4 batches x 2 query-halves (2048 q rows each).
Host rolls z along the sequence axis per core so the kernel's q rows are
always rows 0..2047 (key-order permutation is softmax-invariant).

v7 design notes:
- Host pre-casts to bf16, pre-transposes, and lays every input out as
  the exact SBUF partition image [128, ...] so DMA descriptors carry
  large contiguous segments (small-segment loads are descriptor-bound
  at ~40ns/descriptor).
- k-projection eliminated algebraically: with p = gated q split into
  (p_r, p_i), scores = u_r . z_r^T + u_i . z_i^T where
  u_r = p_r Wkr + p_i Wki, u_i = p_i Wkr - p_r Wki (q rows only).
- v-projection eliminated the same way on the output side: the AV step
  accumulates out1 = z^T e (same matmul count as attn @ v), and a tiny
  per-q-block post-projection out = out1^T [Wv...] replaces projecting
  v over all 4096 keys: 64 matmuls instead of 128.
- Softmax denominators: DVE accumulates E_sum += e per key-chunk; one
  4-matmul partition-reduction per q-block.
- Projection chunks are interleaved with the attention q-block loops
  (chunk i right before q-block i) so the kernel needs only ~2.8MB of
  input before compute starts; the rest streams in under compute.
- q-block tails (denominators + post-projection + normalize + store)
  are deferred into the next q-block's key loop; out1 PSUM banks are
  freed via copies to SBUF.  One shared 3-buffer PSUM pool serves
  projection psums, score psums, and post-projection psums.
- A matmul warmup burst on zeroed SBUF runs during the DMA preamble so
  the PE's HAM clock-gate is at 8/8 when real work arrives.
- Output is written bf16 as a [128, 16, 512] partition image (pairs of
  row-blocks merged per DMA); host unpacks/casts to the f32 result.
"""

import numpy as np
import ml_dtypes

import concourse.bass as bass
import concourse.mybir as mybir
import concourse.tile as tile
from concourse import bacc, bass_utils

F32 = mybir.dt.float32
BF16 = mybir.dt.bfloat16
BF16NP = ml_dtypes.bfloat16

B, S, D = 4, 4096, 256
D2 = 2 * D          # 512
SQ = S // 2         # 2048 q rows per core
SCALE = D ** (-0.5)
KC = S // 128       # 32 key chunks
QB = SQ // 512      # 4 q blocks of 512


def _build():
    nc = bacc.Bacc("TRN2")
    # All inputs are host-prepped bf16 SBUF partition images.
    z_img = nc.dram_tensor("z_img", [128, 4, S], BF16, kind="ExternalInput")
    zr_img = nc.dram_tensor("zr_img", [128, KC, D2], BF16,
                            kind="ExternalInput")
    ctx_img = nc.dram_tensor("ctx_img", [128, 4, SQ], BF16,
                             kind="ExternalInput")
    wq_img = nc.dram_tensor("wq_img", [128, 3, 2, D], BF16,
                            kind="ExternalInput")
    wk_img = nc.dram_tensor("wk_img", [128, 3, 2, D], BF16,
                            kind="ExternalInput")
    wv_img = nc.dram_tensor("wv_img", [128, 2, 2, D2], BF16,
                            kind="ExternalInput")
    wc_img = nc.dram_tensor("wc_img", [128, 4, D2], BF16,
                            kind="ExternalInput")
    bc_img = nc.dram_tensor("bc_img", [128, 4], F32, kind="ExternalInput")
    out = nc.dram_tensor("out", [128, 16, D2], BF16, kind="ExternalOutput")

    mm = nc.tensor.matmul
    QR, QI, QIN = 0, 1, 2    # wq_img slots
    KR, KI, KIN = 0, 1, 2    # wk_img slots
    VZR, VZI = 0, 1          # wv_img slots

    with tile.TileContext(nc) as tc:
        with tc.tile_pool(name="singles", bufs=1) as singles:
            ones = singles.tile([128, 1], BF16, tag="ones")
            nc.vector.memset(ones, 1.0)

            zT = singles.tile([128, 4, S], BF16, tag="zT")
            zR = singles.tile([128, KC, D2], BF16, tag="zR")
            ctxT = singles.tile([128, 4, SQ], BF16, tag="ctxT")
            qTg = singles.tile([128, 4, SQ], BF16, tag="qTg")
            uT = singles.tile([128, 4, SQ], BF16, tag="uT")

            # chunk-0 inputs first so the PE starts ASAP; the rest
            # streams in underneath compute.
            wc = singles.tile([128, 4, D2], BF16, tag="wc")
            nc.sync.dma_start(out=wc, in_=wc_img[:])
            bcT = singles.tile([128, 4], F32, tag="bcT")
            nc.sync.dma_start(out=bcT, in_=bc_img[:])
            for di in range(4):
                nc.sync.dma_start(
                    out=ctxT[:, di, 0:512], in_=ctx_img[:, di, 0:512])
                nc.sync.dma_start(
                    out=zT[:, di, 0:512], in_=z_img[:, di, 0:512])
            wq = singles.tile([128, 3, 2, D], BF16, tag="wq")
            nc.sync.dma_start(out=wq, in_=wq_img[:])
            wk = singles.tile([128, 3, 2, D], BF16, tag="wk")
            nc.sync.dma_start(out=wk, in_=wk_img[:])
            wv = singles.tile([128, 2, 2, D2], BF16, tag="wv")
            nc.sync.dma_start(out=wv, in_=wv_img[:])
            for g in range(2):
                nc.sync.dma_start(
                    out=zR[:, g * 8:(g + 1) * 8, :],
                    in_=zr_img[:, g * 8:(g + 1) * 8, :])
            for di in range(4):
                nc.sync.dma_start(
                    out=ctxT[:, di, 512:2048], in_=ctx_img[:, di, 512:2048])
                nc.sync.dma_start(
                    out=zT[:, di, 512:2048], in_=z_img[:, di, 512:2048])
            for di in range(4):
                nc.sync.dma_start(
                    out=zT[:, di, 2048:4096], in_=z_img[:, di, 2048:4096])
            for g in range(2, 4):
                nc.sync.dma_start(
                    out=zR[:, g * 8:(g + 1) * 8, :],
                    in_=zr_img[:, g * 8:(g + 1) * 8, :])

            with (
                tc.tile_pool(name="gsb", bufs=2) as gsb,
                tc.tile_pool(name="esb", bufs=3) as esb,
                tc.tile_pool(name="osb", bufs=2) as osb,
                tc.tile_pool(name="rcp", bufs=2) as rcp,
                tc.tile_pool(name="esum", bufs=2) as esump,
                tc.tile_pool(name="o1sb", bufs=8) as o1sb,
                tc.tile_pool(name="sps", bufs=3, space="PSUM") as sps,
                tc.tile_pool(name="o1p", bufs=4, space="PSUM") as o1p,
                tc.tile_pool(name="smp", bufs=1, space="PSUM") as smp,
            ):
                # HAM warmup: harmless matmuls on zeroed SBUF while the
                # input DMAs land, so real matmuls start at full clock.
                wu = singles.tile([128, 512], BF16, tag="wu")
                nc.vector.memset(wu, 0)
                for _ in range(20):
                    wup = sps.tile([128, 512], F32, tag="sp")
                    mm(wup, wu[:, 0:128], wu, start=True, stop=True)

                def emit_chunk(sc):
                    """gate/q/u projections for q rows sc*512..+512."""
                    r0 = sc * 512
                    for j in range(4):
                        gp = sps.tile([128, 512], F32, tag="sp")
                        for di in range(4):
                            mm(gp, wc[:, di, j * 128:(j + 1) * 128],
                               ctxT[:, di, r0:r0 + 512], start=(di == 0),
                               stop=(di == 3))
                        gate = gsb.tile([128, 512], F32, tag="gate")
                        nc.scalar.activation(
                            out=gate, in_=gp,
                            func=mybir.ActivationFunctionType.Sigmoid,
                            bias=bcT[:, j:j + 1], scale=1.0)
                        qp = sps.tile([128, 512], F32, tag="sp")
                        jj = j % 2
                        if j < 2:   # q_r^T = Wqr z_r^T - Wqi z_i^T
                            terms = [(QR, 0), (QIN, 2)]
                        else:       # q_i^T = Wqr z_i^T + Wqi z_r^T
                            terms = [(QR, 2), (QI, 0)]
                        n = 0
                        for widx, zoff in terms:
                            for dd in range(2):
                                mm(qp,
                                   wq[:, widx, dd, jj * 128:(jj + 1) * 128],
                                   zT[:, zoff + dd, r0:r0 + 512],
                                   start=(n == 0), stop=(n == 3))
                                n += 1
                        nc.vector.tensor_mul(
                            out=qTg[:, j, r0:r0 + 512], in0=qp, in1=gate)

                    # u^T for this q chunk (folds Wk into q side):
                    #   u_r = p_r Wkr + p_i Wki ; u_i = p_i Wkr - p_r Wki
                    for di in range(4):
                        up = sps.tile([128, 512], F32, tag="sp")
                        jj = di % 2
                        if di < 2:
                            terms = [(KR, 0), (KI, 2)]
                        else:
                            terms = [(KR, 2), (KIN, 0)]
                        n = 0
                        for widx, qoff in terms:
                            for a in range(2):
                                mm(up,
                                   wk[:, widx, a, jj * 128:(jj + 1) * 128],
                                   qTg[:, qoff + a, r0:r0 + 512],
                                   start=(n == 0), stop=(n == 3))
                                n += 1
                        nc.vector.tensor_copy(
                            out=uT[:, di, r0:r0 + 512], in_=up)

                def emit_tail(qb, o1s, esb16):
                    # softmax denominators: one partition-reduction per qb.
                    # start=True clears has_written bank-wide, so only the
                    # first matmul starts; later columns' first writes rely
                    # on cleared bits (overwrite+set).
                    sm = smp.tile([128, 4], F32, tag="sm")
                    for qt in range(4):
                        mm(sm[:, qt:qt + 1],
                           esb16[:, qt * 128:(qt + 1) * 128], ones,
                           start=(qt == 0), stop=True)
                    r = rcp.tile([128, 4], F32, tag="r")
                    nc.vector.reciprocal(out=r, in_=sm)
                    # post-projection: out rows = out1^T [Wv combined],
                    # then normalize by 1/rowsum and store.
                    # out image slot m = qb*4 + 2*(qt%2) + qt//2 pairs the
                    # two DVE-normalized blocks (and the two ACT ones) into
                    # adjacent slots -> one 2KB-per-partition DMA each.
                    o_ev = osb.tile([128, 2, D2], BF16, tag="o_ev")
                    o_od = osb.tile([128, 2, D2], BF16, tag="o_od")
                    for qt in range(4):
                        prj = sps.tile([128, 512], F32, tag="sp")
                        n = 0
                        for dd in range(2):
                            mm(prj, o1s[dd][:, qt * 128:(qt + 1) * 128],
                               wv[:, VZR, dd, :], start=(n == 0),
                               stop=(n == 3))
                            n += 1
                            mm(prj, o1s[2 + dd][:, qt * 128:(qt + 1) * 128],
                               wv[:, VZI, dd, :], start=(n == 0),
                               stop=(n == 3))
                            n += 1
                        dst = (o_ev if qt % 2 == 0 else o_od)[:, qt // 2, :]
                        if qt % 2 == 0:
                            nc.vector.tensor_scalar_mul(
                                out=dst, in0=prj, scalar1=r[:, qt:qt + 1])
                        else:
                            nc.scalar.activation(
                                out=dst, in_=prj,
                                func=mybir.ActivationFunctionType.Copy,
                                scale=r[:, qt:qt + 1])
                    nc.sync.dma_start(
                        out=out[:, qb * 4:qb * 4 + 2, :], in_=o_ev)
                    nc.scalar.dma_start(
                        out=out[:, qb * 4 + 2:qb * 4 + 4, :], in_=o_od)

                pending = None
                for qb in range(QB):
                    emit_chunk(qb)
                    out1 = [o1p.tile([128, D2], F32, tag="out1", name="out1")
                            for _ in range(4)]
                    es = esump.tile([128, 512], F32, tag="es")
                    esb16 = esump.tile([128, 512], BF16, tag="esb16")
                    for kc in range(KC):
                        sp = sps.tile([128, 512], F32, tag="sp")
                        for di in range(4):
                            mm(sp, zT[:, di, kc * 128:(kc + 1) * 128],
                               uT[:, di, qb * 512:(qb + 1) * 512],
                               start=(di == 0), stop=(di == 3))
                        e = esb.tile([128, 512], BF16, tag="e")
                        nc.scalar.activation(
                            out=e, in_=sp,
                            func=mybir.ActivationFunctionType.Exp,
                            scale=float(SCALE))
                        for dc in range(4):
                            mm(out1[dc], zR[:, kc, dc * 128:(dc + 1) * 128],
                               e, start=(kc == 0), stop=(kc == KC - 1))
                        if kc == 0:
                            nc.vector.tensor_copy(out=es, in_=e)
                        elif kc == KC - 1:
                            # last add emits the bf16 copy for the matmul
                            nc.vector.tensor_add(out=esb16, in0=es, in1=e)
                        else:
                            nc.vector.tensor_add(out=es, in0=es, in1=e)
                        if kc == 2 and pending is not None:
                            emit_tail(*pending)
                    # free out1 PSUM banks: copy to SBUF (bf16) for the
                    # post-projection, split across DVE and ACT.
                    o1s = []
                    for dc in range(4):
                        a = o1sb.tile([128, D2], BF16, tag="o1s")
                        if dc % 2 == 0:
                            nc.vector.tensor_copy(out=a, in_=out1[dc])
                        else:
                            nc.scalar.activation(
                                out=a, in_=out1[dc],
                                func=mybir.ActivationFunctionType.Copy)
                        o1s.append(a)
                    pending = (qb, o1s, esb16)
                emit_tail(*pending)

    nc.finalize()
    return nc


_NC_CACHE = {}


def _img(m):
    """[X*128, Y] f32 -> bf16 SBUF partition image [128, X, Y]."""
    x, y = m.shape
    return np.ascontiguousarray(
        m.reshape(x // 128, 128, y).transpose(1, 0, 2)).astype(BF16NP)


def kernel(z_real, z_imag, context, wq_r, wq_i, wk_r, wk_i, wv_r, wv_i,
           wc, bc, _trace=False, _mm_dt=None):
    if "v7" not in _NC_CACHE:
        _NC_CACHE["v7"] = _build()
    nc = _NC_CACHE["v7"]

    z_real = np.asarray(z_real, dtype=np.float32)
    z_imag = np.asarray(z_imag, dtype=np.float32)
    context = np.asarray(context, dtype=np.float32)
    f32 = lambda x: np.ascontiguousarray(np.asarray(x, dtype=np.float32))
    wq_r, wq_i = f32(wq_r), f32(wq_i)
    wk_r, wk_i = f32(wk_r), f32(wk_i)
    wv_r, wv_i = f32(wv_r), f32(wv_i)
    wc_, bc_ = f32(wc), f32(bc)

    ws = {
        "wq_img": np.stack(
            [_img(wq_r.T), _img(wq_i.T), _img(-wq_i.T)], axis=1),
        "wk_img": np.stack(
            [_img(wk_r), _img(wk_i), _img(-wk_i)], axis=1),
        "wv_img": np.stack(
            [_img(np.ascontiguousarray(np.concatenate(
                [wv_r.T, wv_i.T], axis=1))),
             _img(np.ascontiguousarray(np.concatenate(
                 [-wv_i.T, wv_r.T], axis=1)))], axis=1),
        "wc_img": _img(np.ascontiguousarray(wc_.T)),
        "bc_img": np.ascontiguousarray(
            bc_.reshape(4, 128).T),
    }

    in_maps = []
    for c in range(8):
        b, h = c // 2, c % 2
        zr = np.roll(z_real[b], -h * SQ, axis=0)
        zi = np.roll(z_imag[b], -h * SQ, axis=0)
        cx = context[b, h * SQ:(h + 1) * SQ]
        in_maps.append({
            "z_img": np.concatenate(
                [_img(np.ascontiguousarray(zr.T)),
                 _img(np.ascontiguousarray(zi.T))], axis=1),
            "zr_img": _img(np.concatenate([zr, zi], axis=1)),
            "ctx_img": _img(np.ascontiguousarray(cx.T)),
            **ws,
        })
    res = bass_utils.run_bass_kernel_spmd(
        nc, in_maps, core_ids=list(range(8)), trace=_trace)

    # unpack the out image: slot m = qb*4 + 2*(qt%2) + qt//2
    full = np.empty((B, S, D2), dtype=np.float32)
    for c in range(8):
        b, h = c // 2, c % 2
        img = np.asarray(res.results[c]["out"], dtype=np.float32)
        dst = full[b, h * SQ:(h + 1) * SQ, :]
        for m in range(16):
            qb, u = divmod(m, 4)
            qt = 2 * (u % 2) + u // 2
            i = qb * 4 + qt
            dst[i * 128:(i + 1) * 128, :] = img[:, m, :]
    if _trace:
        return full, res
    return full


# revision 12
# speedup vs baseline: 1.1420x; 1.0020x over previous
"""ContextAwareAttention Trainium2 kernel (v7).

Problem (hardcoded shapes): B=4, S=4096, DIM=256.
  q/k/v = complex linear projections of (z_real, z_imag); q gated by
  sigmoid(context @ wc.T + bc); scores = qf @ kf.T / 16; softmax;
  out = [attn @ v_r, attn @ v_i].

Sharding: 8 cores = 4 batches x 2 query-halves (2048 q rows each).
Host rolls z along the sequence axis per core so the kernel's q rows are
always rows 0..2047 (key-order permutation is softmax-invariant).

v7 design notes:
- Host pre-casts to bf16, pre-transposes, and lays every input out as
  the exact SBUF partition image [128, ...] so DMA descriptors carry
  large contiguous segments (small-segment loads are descriptor-bound
  at ~40ns/descriptor).
- k-projection eliminated algebraically: with p = gated q split into
  (p_r, p_i), scores = u_r . z_r^T + u_i . z_i^T where
  u_r = p_r Wkr + p_i Wki, u_i = p_i Wkr - p_r Wki (q rows only).
- v-projection eliminated the same way on the output side: the AV step
  accumulates out1 = z^T e (same matmul count as attn @ v), and a tiny
  per-q-block post-projection out = out1^T [Wv...] replaces projecting
  v over all 4096 keys: 64 matmuls instead of 128.
- Softmax denominators: DVE accumulates E_sum += e per key-chunk; one
  4-matmul partition-reduction per q-block.
- Projection chunks are interleaved with the attention q-block loops
  (chunk i right before q-block i) so the kernel needs only ~2.8MB of
  input before compute starts; the rest streams in under compute.
- q-block tails (denominators + post-projection + normalize + store)
  are deferred into the next q-block's key loop; out1 PSUM banks are
  freed via copies to SBUF.  One shared 3-buffer PSUM pool serves
  projection psums, score psums, and post-projection psums.
- A matmul warmup burst on zeroed SBUF runs during the DMA preamble so
  the PE's HAM clock-gate is at 8/8 when real work arrives.
- Output is written bf16 as a [128, 16, 512] partition image (pairs of
  row-blocks merged per DMA); host unpacks/casts to the f32 result.
"""

import numpy as np
import ml_dtypes

import concourse.bass as bass
import concourse.mybir as mybir
import concourse.tile as tile
from concourse import bacc, bass_utils

F32 = mybir.dt.float32
BF16 = mybir.dt.bfloat16
BF16NP = ml_dtypes.bfloat16

B, S, D = 4, 4096, 256
D2 = 2 * D          # 512
SQ = S // 2         # 2048 q rows per core
SCALE = D ** (-0.5)
KC = S // 128       # 32 key chunks
QB = SQ // 512      # 4 q blocks of 512


def _build():
    nc = bacc.Bacc("TRN2")
    # All inputs are host-prepped bf16 SBUF partition images.
    z_img = nc.dram_tensor("z_img", [128, 4, S], BF16, kind="ExternalInput")
    zr_img = nc.dram_tensor("zr_img", [128, KC, D2], BF16,
                            kind="ExternalInput")
    ctx_img = nc.dram_tensor("ctx_img", [128, 4, SQ], BF16,
                             kind="ExternalInput")
    wq_img = nc.dram_tensor("wq_img", [128, 3, 2, D], BF16,
                            kind="ExternalInput")
    wk_img = nc.dram_tensor("wk_img", [128, 3, 2, D], BF16,
                            kind="ExternalInput")
    wv_img = nc.dram_tensor("wv_img", [128, 2, 2, D2], BF16,
                            kind="ExternalInput")
    wc_img = nc.dram_tensor("wc_img", [128, 4, D2], BF16,
                            kind="ExternalInput")
    bc_img = nc.dram_tensor("bc_img", [128, 4], F32, kind="ExternalInput")
    out = nc.dram_tensor("out", [128, 16, D2], BF16, kind="ExternalOutput")

    mm = nc.tensor.matmul
    QR, QI, QIN = 0, 1, 2    # wq_img slots
    KR, KI, KIN = 0, 1, 2    # wk_img slots
    VZR, VZI = 0, 1          # wv_img slots

    with tile.TileContext(nc) as tc:
        with tc.tile_pool(name="singles", bufs=1) as singles:
            ones = singles.tile([128, 1], BF16, tag="ones")
            nc.vector.memset(ones, 1.0)

            zT = singles.tile([128, 4, S], BF16, tag="zT")
            zR = singles.tile([128, KC, D2], BF16, tag="zR")
            ctxT = singles.tile([128, 4, SQ], BF16, tag="ctxT")
            qTg = singles.tile([128, 4, SQ], BF16, tag="qTg")
            uT = singles.tile([128, 4, SQ], BF16, tag="uT")

            # chunk-0 inputs first so the PE starts ASAP; the rest
            # streams in underneath compute.
            wc = singles.tile([128, 4, D2], BF16, tag="wc")
            nc.sync.dma_start(out=wc, in_=wc_img[:])
            bcT = singles.tile([128, 4], F32, tag="bcT")
            nc.sync.dma_start(out=bcT, in_=bc_img[:])
            for di in range(4):
                nc.sync.dma_start(
                    out=ctxT[:, di, 0:512], in_=ctx_img[:, di, 0:512])
                nc.sync.dma_start(
                    out=zT[:, di, 0:512], in_=z_img[:, di, 0:512])
            wq = singles.tile([128, 3, 2, D], BF16, tag="wq")
            nc.sync.dma_start(out=wq, in_=wq_img[:])
            wk = singles.tile([128, 3, 2, D], BF16, tag="wk")
            nc.sync.dma_start(out=wk, in_=wk_img[:])
            wv = singles.tile([128, 2, 2, D2], BF16, tag="wv")
            nc.sync.dma_start(out=wv, in_=wv_img[:])
            for g in range(2):
                nc.sync.dma_start(
                    out=zR[:, g * 8:(g + 1) * 8, :],
                    in_=zr_img[:, g * 8:(g + 1) * 8, :])
            for di in range(4):
                nc.sync.dma_start(
                    out=ctxT[:, di, 512:2048], in_=ctx_img[:, di, 512:2048])
                nc.sync.dma_start(
                    out=zT[:, di, 512:2048], in_=z_img[:, di, 512:2048])
            for di in range(4):
                nc.sync.dma_start(
                    out=zT[:, di, 2048:4096], in_=z_img[:, di, 2048:4096])
            for g in range(2, 4):
                nc.sync.dma_start(
                    out=zR[:, g * 8:(g + 1) * 8, :],
                    in_=zr_img[:, g * 8:(g + 1) * 8, :])

            with (
                tc.tile_pool(name="gsb", bufs=2) as gsb,
                tc.tile_pool(name="esb", bufs=3) as esb,
                tc.tile_pool(name="osb", bufs=2) as osb,
                tc.tile_pool(name="rcp", bufs=2) as rcp,
                tc.tile_pool(name="esum", bufs=2) as esump,
                tc.tile_pool(name="o1sb", bufs=8) as o1sb,
                tc.tile_pool(name="sps", bufs=3, space="PSUM") as sps,
                tc.tile_pool(name="o1p", bufs=4, space="PSUM") as o1p,
                tc.tile_pool(name="smp", bufs=1, space="PSUM") as smp,
            ):
                # HAM warmup: harmless matmuls on zeroed SBUF while the
                # input DMAs land, so real matmuls start at full clock.
                wu = singles.tile([128, 512], BF16, tag="wu")
                nc.vector.memset(wu, 0)
                for _ in range(32):
                    wup = sps.tile([128, 512], F32, tag="sp")
                    mm(wup, wu[:, 0:128], wu, start=True, stop=True)

                def emit_chunk(sc):
                    """gate/q/u projections for q rows sc*512..+512."""
                    r0 = sc * 512
                    for j in range(4):
                        gp = sps.tile([128, 512], F32, tag="sp")
                        for di in range(4):
                            mm(gp, wc[:, di, j * 128:(j + 1) * 128],
                               ctxT[:, di, r0:r0 + 512], start=(di == 0),
                               stop=(di == 3))
                        gate = gsb.tile([128, 512], F32, tag="gate")
                        nc.scalar.activation(
                            out=gate, in_=gp,
                            func=mybir.ActivationFunctionType.Sigmoid,
                            bias=bcT[:, j:j + 1], scale=1.0)
                        qp = sps.tile([128, 512], F32, tag="sp")
                        jj = j % 2
                        if j < 2:   # q_r^T = Wqr z_r^T - Wqi z_i^T
                            terms = [(QR, 0), (QIN, 2)]
                        else:       # q_i^T = Wqr z_i^T + Wqi z_r^T
                            terms = [(QR, 2), (QI, 0)]
                        n = 0
                        for widx, zoff in terms:
                            for dd in range(2):
                                mm(qp,
                                   wq[:, widx, dd, jj * 128:(jj + 1) * 128],
                                   zT[:, zoff + dd, r0:r0 + 512],
                                   start=(n == 0), stop=(n == 3))
                                n += 1
                        nc.vector.tensor_mul(
                            out=qTg[:, j, r0:r0 + 512], in0=qp, in1=gate)

                    # u^T for this q chunk (folds Wk into q side):
                    #   u_r = p_r Wkr + p_i Wki ; u_i = p_i Wkr - p_r Wki
                    for di in range(4):
                        up = sps.tile([128, 512], F32, tag="sp")
                        jj = di % 2
                        if di < 2:
                            terms = [(KR, 0), (KI, 2)]
                        else:
                            terms = [(KR, 2), (KIN, 0)]
                        n = 0
                        for widx, qoff in terms:
                            for a in range(2):
                                mm(up,
                                   wk[:, widx, a, jj * 128:(jj + 1) * 128],
                                   qTg[:, qoff + a, r0:r0 + 512],
                                   start=(n == 0), stop=(n == 3))
                                n += 1
                        nc.vector.tensor_copy(
                            out=uT[:, di, r0:r0 + 512], in_=up)

                def emit_tail(qb, o1s, esb16):
                    # softmax denominators: one partition-reduction per qb.
                    # start=True clears has_written bank-wide, so only the
                    # first matmul starts; later columns' first writes rely
                    # on cleared bits (overwrite+set).
                    sm = smp.tile([128, 4], F32, tag="sm")
                    for qt in range(4):
                        mm(sm[:, qt:qt + 1],
                           esb16[:, qt * 128:(qt + 1) * 128], ones,
                           start=(qt == 0), stop=True)
                    r = rcp.tile([128, 4], F32, tag="r")
                    nc.vector.reciprocal(out=r, in_=sm)
                    # post-projection: out rows = out1^T [Wv combined],
                    # then normalize by 1/rowsum and store.
                    # out image slot m = qb*4 + 2*(qt%2) + qt//2 pairs the
                    # two DVE-normalized blocks (and the two ACT ones) into
                    # adjacent slots -> one 2KB-per-partition DMA each.
                    o_ev = osb.tile([128, 2, D2], BF16, tag="o_ev")
                    o_od = osb.tile([128, 2, D2], BF16, tag="o_od")
                    for qt in range(4):
                        prj = sps.tile([128, 512], F32, tag="sp")
                        n = 0
                        for dd in range(2):
                            mm(prj, o1s[dd][:, qt * 128:(qt + 1) * 128],
                               wv[:, VZR, dd, :], start=(n == 0),
                               stop=(n == 3))
                            n += 1
                            mm(prj, o1s[2 + dd][:, qt * 128:(qt + 1) * 128],
                               wv[:, VZI, dd, :], start=(n == 0),
                               stop=(n == 3))
                            n += 1
                        dst = (o_ev if qt % 2 == 0 else o_od)[:, qt // 2, :]
                        if qt % 2 == 0:
                            nc.vector.tensor_scalar_mul(
                                out=dst, in0=prj, scalar1=r[:, qt:qt + 1])
                        else:
                            nc.scalar.activation(
                                out=dst, in_=prj,
                                func=mybir.ActivationFunctionType.Copy,
                                scale=r[:, qt:qt + 1])
                    nc.sync.dma_start(
                        out=out[:, qb * 4:qb * 4 + 2, :], in_=o_ev)
                    nc.scalar.dma_start(
                        out=out[:, qb * 4 + 2:qb * 4 + 4, :], in_=o_od)

                pending = None
                for qb in range(QB):
                    emit_chunk(qb)
                    out1 = [o1p.tile([128, D2], F32, tag="out1", name="out1")
                            for _ in range(4)]
                    es = esump.tile([128, 512], F32, tag="es")
                    esb16 = esump.tile([128, 512], BF16, tag="esb16")
                    for kc in range(KC):
                        sp = sps.tile([128, 512], F32, tag="sp")
                        for di in range(4):
                            mm(sp, zT[:, di, kc * 128:(kc + 1) * 128],
                               uT[:, di, qb * 512:(qb + 1) * 512],
                               start=(di == 0), stop=(di == 3))
                        e = esb.tile([128, 512], BF16, tag="e")
                        nc.scalar.activation(
                            out=e, in_=sp,
                            func=mybir.ActivationFunctionType.Exp,
                            scale=float(SCALE))
                        for dc in range(4):
                            mm(out1[dc], zR[:, kc, dc * 128:(dc + 1) * 128],
                               e, start=(kc == 0), stop=(kc == KC - 1))
                        if kc == 0:
                            nc.vector.tensor_copy(out=es, in_=e)
                        elif kc == KC - 1:
                            # last add emits the bf16 copy for the matmul
                            nc.vector.tensor_add(out=esb16, in0=es, in1=e)
                        else:
                            nc.vector.tensor_add(out=es, in0=es, in1=e)
                        if kc == 2 and pending is not None:
                            emit_tail(*pending)
                    # free out1 PSUM banks: copy to SBUF (bf16) for the
                    # post-projection, split across DVE and ACT.
                    o1s = []
                    for dc in range(4):
                        a = o1sb.tile([128, D2], BF16, tag="o1s")
                        if dc % 2 == 0:
                            nc.vector.tensor_copy(out=a, in_=out1[dc])
                        else:
                            nc.scalar.activation(
                                out=a, in_=out1[dc],
                                func=mybir.ActivationFunctionType.Copy)
                        o1s.append(a)
                    pending = (qb, o1s, esb16)
                emit_tail(*pending)

    nc.finalize()
    return nc


_NC_CACHE = {}


def _img(m):
    """[X*128, Y] f32 -> bf16 SBUF partition image [128, X, Y]."""
    x, y = m.shape
    return np.ascontiguousarray(
        m.reshape(x // 128, 128, y).transpose(1, 0, 2)).astype(BF16NP)


def kernel(z_real, z_imag, context, wq_r, wq_i, wk_r, wk_i, wv_r, wv_i,
           wc, bc, _trace=False, _mm_dt=None):
    if "v7" not in _NC_CACHE:
        _NC_CACHE["v7"] = _build()
    nc = _NC_CACHE["v7"]

    z_real = np.asarray(z_real, dtype=np.float32)
    z_imag = np.asarray(z_imag, dtype=np.float32)
    context = np.asarray(context, dtype=np.float32)
    f32 = lambda x: np.ascontiguousarray(np.asarray(x, dtype=np.float32))
    wq_r, wq_i = f32(wq_r), f32(wq_i)
    wk_r, wk_i = f32(wk_r), f32(wk_i)
    wv_r, wv_i = f32(wv_r), f32(wv_i)
    wc_, bc_ = f32(wc), f32(bc)

    ws = {
        "wq_img": np.stack(
            [_img(wq_r.T), _img(wq_i.T), _img(-wq_i.T)], axis=1),
        "wk_img": np.stack(
            [_img(wk_r), _img(wk_i), _img(-wk_i)], axis=1),
        "wv_img": np.stack(
            [_img(np.ascontiguousarray(np.concatenate(
                [wv_r.T, wv_i.T], axis=1))),
             _img(np.ascontiguousarray(np.concatenate(
                 [-wv_i.T, wv_r.T], axis=1)))], axis=1),
        "wc_img": _img(np.ascontiguousarray(wc_.T)),
        "bc_img": np.ascontiguousarray(
            bc_.reshape(4, 128).T),
    }

    in_maps = []
    for c in range(8):
        b, h = c // 2, c % 2
        zr = np.roll(z_real[b], -h * SQ, axis=0)
        zi = np.roll(z_imag[b], -h * SQ, axis=0)
        cx = context[b, h * SQ:(h + 1) * SQ]
        in_maps.append({
            "z_img": np.concatenate(
                [_img(np.ascontiguousarray(zr.T)),
                 _img(np.ascontiguousarray(zi.T))], axis=1),
            "zr_img": _img(np.concatenate([zr, zi], axis=1)),
            "ctx_img": _img(np.ascontiguousarray(cx.T)),
            **ws,
        })
    res = bass_utils.run_bass_kernel_spmd(
        nc, in_maps, core_ids=list(range(8)), trace=_trace)

    # unpack the out image: slot m = qb*4 + 2*(qt%2) + qt//2
    full = np.empty((B, S, D2), dtype=np.float32)
    for c in range(8):
        b, h = c // 2, c % 2
        img = np.asarray(res.results[c]["out"], dtype=np.float32)
        dst = full[b, h * SQ:(h + 1) * SQ, :]
        for m in range(16):
            qb, u = divmod(m, 4)
            qt = 2 * (u % 2) + u // 2
            i = qb * 4 + qt
            dst[i * 128:(i + 1) * 128, :] = img[:, m, :]
    if _trace:
        return full, res
    return full
